# revision 9
# baseline (speedup 1.0000x reference)
"""F1-score (macro) kernel for Trainium2, 8 NeuronCores.

Layout: per core (data-parallel over rows), rows are partition-contiguous:
local row = p*J + j  (p in [0,128), j in [0,976)), 72-row tail handled flat.
Tiles of TK=61 j-columns: xh [128, 61, 128] bf16, cast f32->bf16 during the
SWDGE DMA itself (HBM reads stay f32 = the memory roofline; SBUF and all
compute go 16-bit for the DVE 2x/4x perf modes).

Per tile:
  - DVE: oht[:,j,:] = (iota == t[:, j])       tensor_scalar is_equal, 4x bf16
  - DVE: rowmax via 4-stage tensor_tensor max tree (2x bf16) + small reduce
  - DVE: ohp[:,j,:] = (xh == mh)  for j in DVE chunks (is_equal, 4x)
  - ACT: ohp[:,j,:] = sign(mh - xh) (= anti) for j in ACT chunks
  - PE : acc_p += oht_j^T @ ohp_j (DVE chunks), acc_a += oht_j^T @ anti_j
Host: cm = sum_cores [acc_p + (support_act - acc_a)]; macro-F1 epilogue.

bf16 tie semantics (multi-hot on exact bf16 ties) verified on the harness
data: rel err 7.7e-4 << 2e-2.
"""

import sys
import time

if "/opt/trn_rl_repo" not in sys.path:
    sys.path.insert(0, "/opt/trn_rl_repo")

import numpy as np

import concourse.bacc as bacc
import concourse.mybir as mybir
import concourse.tile as tile
from concourse import bass_utils

C = 128
N = 1_000_000
NCORES = 8
R = N // NCORES          # 125000 rows per core
J = 976                  # j-columns per partition (128*976 = 124928 rows)
TK = 61                  # j-columns per tile
NT = J // TK             # 16 tiles
TAIL = R - 128 * J       # 72 leftover rows
EPS = 1e-12

N_ACT = 52               # ohp chunks per tile computed on ACT (Sign/anti form)
ACT_SET = tuple(range(TK - N_ACT, TK))   # j-locals assigned to ACT
GS_OHT_SET = (29, 59)    # oht chunks per tile computed on GpSimd

_CACHE = {}


def _build():
    f32 = mybir.dt.float32
    bf16 = mybir.dt.bfloat16
    Alu = mybir.AluOpType
    Act = mybir.ActivationFunctionType

    nc = bacc.Bacc("TRN2", target_bir_lowering=False, debug=False,
                   num_devices=NCORES)
    yp = nc.dram_tensor("yp", [R, C], f32, kind="ExternalInput")
    yt = nc.dram_tensor("yt", [R], f32, kind="ExternalInput")
    cm = nc.dram_tensor("cm", [C, 2 * C], f32, kind="ExternalOutput")

    yp_grid = yp.ap()[0 : 128 * J, :].rearrange("(p j) c -> p j c", p=128)
    yt_grid = yt.ap()[0 : 128 * J].rearrange("(p j) -> p j", p=128)

    with tile.TileContext(nc) as tc:
        with (
            tc.tile_pool(name="const", bufs=1) as cpool,
            tc.tile_pool(name="xin", bufs=3) as xpool,
            tc.tile_pool(name="tree", bufs=1) as tpool,
            tc.tile_pool(name="oh", bufs=3) as ohpool,
            tc.tile_pool(name="small", bufs=3) as spool,
            tc.tile_pool(name="psum", bufs=1, space="PSUM") as psum,
        ):
            iota_i = cpool.tile([128, C], mybir.dt.int32)
            nc.gpsimd.iota(iota_i[:], pattern=[[1, C]], base=0,
                           channel_multiplier=0)
            iota_h = cpool.tile([128, C], bf16)
            nc.vector.tensor_copy(iota_h[:], iota_i[:])

            # whole y_true for the grid (f32: tensor_scalar scalars must be f32)
            t_all = cpool.tile([128, J], f32)
            nc.sync.dma_start(t_all[:], yt_grid)
            t_tail = cpool.tile([TAIL, 1], f32)
            nc.sync.dma_start(
                t_tail[:], yt.ap()[128 * J : R].rearrange("(p k) -> p k", k=1)
            )

            acc_p = psum.tile([C, C], f32)
            acc_a = psum.tile([C, C], f32)
            state = {"p": False, "a": False}
            n_a_total = NT * N_ACT

            def emit_tile(i):
                j0 = i * TK
                xh = xpool.tile([128, TK, C], bf16, tag="xh")
                nc.gpsimd.dma_start(xh[:], yp_grid[:, j0 : j0 + TK, :])

                oht = ohpool.tile([128, TK, C], bf16, tag="oht")
                for j in range(TK):
                    eng = nc.gpsimd if j in GS_OHT_SET else nc.vector
                    eng.tensor_scalar(
                        oht[:, j, :], iota_h[:], t_all[:, j0 + j : j0 + j + 1],
                        None, op0=Alu.is_equal,
                    )

                # rowmax tree: 64 -> 32 -> 16 -> 8 then reduce
                m1 = tpool.tile([128, TK, 64], bf16, tag="m1")
                nc.vector.tensor_tensor(
                    m1[:], xh[:, :, 0:64], xh[:, :, 64:128], op=Alu.max
                )
                m2 = tpool.tile([128, TK, 32], bf16, tag="m2")
                nc.vector.tensor_tensor(
                    m2[:], m1[:, :, 0:32], m1[:, :, 32:64], op=Alu.max
                )
                m3 = tpool.tile([128, TK, 16], bf16, tag="m3")
                nc.vector.tensor_tensor(
                    m3[:], m2[:, :, 0:16], m2[:, :, 16:32], op=Alu.max
                )
                m4 = tpool.tile([128, TK, 8], bf16, tag="m4")
                nc.vector.tensor_tensor(
                    m4[:], m3[:, :, 0:8], m3[:, :, 8:16], op=Alu.max
                )
                m5 = tpool.tile([128, TK, 4], bf16, tag="m5")
                nc.vector.tensor_tensor(
                    m5[:], m4[:, :, 0:4], m4[:, :, 4:8], op=Alu.max
                )
                m6 = tpool.tile([128, TK, 2], bf16, tag="m6")
                nc.vector.tensor_tensor(
                    m6[:], m5[:, :, 0:2], m5[:, :, 2:4], op=Alu.max
                )
                mh = spool.tile([128, TK], f32, tag="mh")
                nc.vector.tensor_tensor(
                    mh[:, :, None], m6[:, :, 0:1], m6[:, :, 1:2], op=Alu.max
                )

                ohp = ohpool.tile([128, TK, C], bf16, tag="ohp")
                for j in range(TK):
                    if j in ACT_SET:
                        nc.scalar.activation(
                            ohp[:, j, :], xh[:, j, :], Act.Sign,
                            bias=mh[:, j : j + 1], scale=-1.0,
                        )
                        acc, key = acc_a, "a"
                    else:
                        nc.vector.tensor_scalar(
                            ohp[:, j, :], xh[:, j, :], mh[:, j : j + 1],
                            None, op0=Alu.is_equal,
                        )
                        acc, key = acc_p, "p"
                    nc.tensor.matmul(
                        acc[:], oht[:, j, :], ohp[:, j, :],
                        start=not state[key], stop=False,
                    )
                    state[key] = True

            for i in range(NT):
                emit_tile(i)

            # mark end of acc_a accumulation with a zero-contribution matmul?
            # Instead: reuse last ACT matmul as stop by emitting tail first is
            # complex; simply do a final stop matmul on acc_a with zero rows is
            # not possible -- use stop on a redundant matmul of zeros.
            # Simpler: tail goes to acc_p with stop=True, and acc_a gets its
            # stop flag via a final 1-row matmul of zeros.

            # tail rows (72)
            xt = xpool.tile([TAIL, 1, C], bf16, tag="xtail")
            nc.gpsimd.dma_start(
                xt[:],
                yp.ap()[128 * J : R, :].rearrange("(p k) c -> p k c", k=1),
            )
            mh_t = spool.tile([TAIL, 1], f32, tag="mhtail")
            nc.vector.tensor_reduce(
                mh_t[:], xt[:], axis=mybir.AxisListType.X, op=Alu.max
            )
            ohp_t = ohpool.tile([TAIL, C], bf16, tag="ohptail")
            oht_t = ohpool.tile([TAIL, C], bf16, tag="ohttail")
            nc.vector.tensor_scalar(
                ohp_t[:], xt[:, 0, :], mh_t[:], None, op0=Alu.is_equal
            )
            nc.vector.tensor_scalar(
                oht_t[:], iota_h[:TAIL, :], t_tail[:], None, op0=Alu.is_equal
            )
            nc.tensor.matmul(
                acc_p[:], oht_t[:], ohp_t[:], start=False, stop=True
            )
            # close acc_a accumulation: repeat the last tail matmul shape into
            # acc_a with zero operands? Use oht_t row0 x zero vector instead.
            zrow = cpool.tile([1, C], bf16)
            nc.vector.memset(zrow[:], 0.0)
            nc.tensor.matmul(
                acc_a[:], zrow[:], zrow[:], start=False, stop=True
            )

            out_sb = spool.tile([C, 2 * C], f32, tag="out")
            nc.scalar.copy(out_sb[:, 0:C], acc_p[:])
            nc.scalar.copy(out_sb[:, C : 2 * C], acc_a[:])
            nc.sync.dma_start(cm.ap()[:], out_sb[:])

    nc.compile()
    return nc


def _get_nc():
    if "nc" not in _CACHE:
        _CACHE["nc"] = _build()
    return _CACHE["nc"]


def _act_row_mask():
    """Bool mask over local rows [0, R): rows whose chunk went to ACT."""
    jl = np.arange(J) % TK
    jmask = np.isin(jl, np.asarray(ACT_SET))
    mask = np.zeros(R, dtype=bool)
    mask[: 128 * J] = np.broadcast_to(jmask, (128, J)).ravel()
    return mask


def _run(y_pred, y_true, trace=False):
    nc = _get_nc()
    y_pred = np.ascontiguousarray(np.asarray(y_pred, dtype=np.float32))
    yt_i = np.asarray(y_true).astype(np.int64)
    yt_f = yt_i.astype(np.float32)
    in_maps = [
        {
            "yp": y_pred[c * R : (c + 1) * R],
            "yt": np.ascontiguousarray(yt_f[c * R : (c + 1) * R]),
        }
        for c in range(NCORES)
    ]
    res = None
    for attempt in range(3):
        try:
            res = bass_utils.run_bass_kernel_spmd(
                nc, in_maps, core_ids=list(range(NCORES)), trace=trace
            )
            break
        except Exception:
            if attempt == 2:
                raise
            time.sleep(2.0)
    amask = _act_row_mask()
    cm_total = np.zeros((C, C), dtype=np.float64)
    for c, r in enumerate(res.results):
        out = r["cm"].astype(np.float64)
        acc_p, acc_a = out[:, 0:C], out[:, C : 2 * C]
        yt_core = yt_i[c * R : (c + 1) * R]
        support_act = np.bincount(yt_core[amask], minlength=C).astype(
            np.float64
        )
        cm_total += acc_p + (support_act[:, None] - acc_a)
    diag = np.diagonal(cm_total)
    precision = diag / (cm_total.sum(axis=1) + EPS)
    recall = diag / (cm_total.sum(axis=0) + EPS)
    f1 = 2.0 * precision * recall / (precision + recall + EPS)
    return np.float32(f1.mean()), res


def kernel(y_pred, y_true):
    out, _ = _run(y_pred, y_true, trace=False)
    return out
